# revision 26
# baseline (speedup 1.0000x reference)
"""DeepSeek-V3 MLA attention wrapper kernel for 8 Trainium2 NeuronCores.

Sharding: core c = (b, qh) with b = c // 2 (batch index), qh = c % 2 (token
half). Each core computes the compressed KV (kv_a down-proj + rope'd k_pe)
for its OWN 1024-token half, then the two cores of a batch exchange just the
compressed c_kv/k_pe (1.2 MB) with a 2-core AllGather. Each core then runs
the kv_b up-projection for the FULL sequence locally, keeping k_nope/v
entirely in SBUF (never spilled to HBM). Q path + attention + output
projection cover the core's own 1024 query rows; per-core outputs are
disjoint row blocks the host concats.

Key design points:
- All weights/activations bf16 (fp32 PSUM accumulate); panelized weight
  layouts so weights stream as a few large contiguous DMAs on sync.
- Only the tiny AllGather and its consumers live on gpsimd, so weight
  streaming never queues behind a collective wait.
- Attention processes head pairs (2j, 2j+1): their 64-row rope matmuls
  go to disjoint PE row-groups (partitions 0:64 / 64:128) and run
  concurrently in the array.
- Softmax row-sums: a selector-matrix matmul drops each head's key-sum into
  row h of a [16, TQ] accumulator; one batched DVE reciprocal; the 1/Z
  broadcast is another selector matmul. Normalization is deferred to after
  the attention loop (p*v stays unnormalized in PSUM/bf16 meanwhile).
"""

import sys

sys.path.insert(0, "/opt/trn_rl_repo")

import numpy as np

import concourse.tile as tile
from concourse import bacc, mybir

B, S, H = 4, 2048, 16
HID = 2048
Q_LORA = 1536
KV_LORA = 512
D_NOPE, D_ROPE, D_V = 128, 64, 128
D_QK = D_NOPE + D_ROPE  # 192
THETA = 10000.0
EPS = 1e-6

P = 128
TQ = S // 2  # tokens per core (queries); own KV half
NT = 512  # matmul free-dim tile
NQT = TQ // NT  # 2
N_CORES = 8
PAIRS = [[0, 1], [2, 3], [4, 5], [6, 7]]

NKH = HID // P  # 16
NKQ = Q_LORA // P  # 12
NKV = KV_LORA // P  # 4
NKO = H * D_V // P  # 16
NJ = H // 2  # 8
KVA_COLS = KV_LORA + P  # 512 ckv | pe(64)+pad(64)
QBJ_COLS = 3 * P  # per-j: nope0 | nope1 | [pe0|pe1]

F32 = mybir.dt.float32
F32R = mybir.dt.float32r
BF16 = mybir.dt.bfloat16
EXP = mybir.ActivationFunctionType.Exp
SQRT = mybir.ActivationFunctionType.Sqrt
COPY = mybir.ActivationFunctionType.Copy

SCALE = float(D_QK) ** -0.5

_CACHE = {}


def build_nc():
    import os

    REP = int(os.environ.get("KREPEAT", "1"))
    key = ("nc", REP)
    if key in _CACHE:
        return _CACHE[key]
    nc = bacc.Bacc(None, target_bir_lowering=False, num_devices=N_CORES)

    xq = nc.dram_tensor("xq", [P, NKH * TQ], BF16, kind="ExternalInput")
    wkva = nc.dram_tensor("wkva", [P, NKH, KVA_COLS], BF16, kind="ExternalInput")
    wvb_v = nc.dram_tensor("wvb_v", [P, 4, NKV, NT], BF16, kind="ExternalInput")
    wvb_k = nc.dram_tensor("wvb_k", [P, H, NKV, P], BF16, kind="ExternalInput")
    wqa = nc.dram_tensor("wqa", [P, NKQ, NKH * P], BF16, kind="ExternalInput")
    wqb = nc.dram_tensor("wqb", [P, NJ, NKQ * QBJ_COLS], BF16, kind="ExternalInput")
    wo = nc.dram_tensor("wo", [P, 4, NKO * NT], BF16, kind="ExternalInput")
    cosq = nc.dram_tensor("cosq", [P, TQ], BF16, kind="ExternalInput")
    sinq = nc.dram_tensor("sinq", [P, TQ], BF16, kind="ExternalInput")
    cosk = nc.dram_tensor("cosk", [D_ROPE, TQ], BF16, kind="ExternalInput")
    sink = nc.dram_tensor("sink", [D_ROPE, TQ], BF16, kind="ExternalInput")
    ones_in = nc.dram_tensor("ones", [P, 1], F32R, kind="ExternalInput")
    selcol_in = nc.dram_tensor("selcol", [P, H * H], BF16, kind="ExternalInput")
    selbrd_in = nc.dram_tensor("selbrd", [H, H * P], F32R, kind="ExternalInput")
    outT = nc.dram_tensor("outT", [HID, TQ], F32, kind="ExternalOutput")

    # compressed-KV exchange buffers (HBM): 4 ckv slabs + 1 k_pe slab
    ckx_loc = nc.dram_tensor("ckx_loc", [NKV + 1, P, TQ], BF16, kind="Internal")
    ckx_gth = nc.dram_tensor("ckx_gth", [2, NKV + 1, P, TQ], BF16, kind="Internal")

    with tile.TileContext(nc) as tc:
        with tc.tile_pool(name="const", bufs=1) as const:
            ones_row = const.tile([1, P], F32R, name="ones_row")
            nc.sync.dma_start(ones_row, ones_in[:, :].rearrange("p one -> one p"))
            ones_col_bf = const.tile([P, 1], BF16, name="ones_col_bf")
            nc.vector.memset(ones_col_bf, 1.0)
            eps_t = const.tile([1, 1], F32, name="eps_t")
            nc.vector.memset(eps_t, EPS)
            selcol = const.tile([P, H * H], BF16, name="selcol")
            nc.sync.dma_start(selcol, selcol_in[:, :])
            selbrd = const.tile([H, H * P], F32R, name="selbrd")
            nc.sync.dma_start(selbrd, selbrd_in[:, :])

            for rep in range(REP):
                with (
                    tc.tile_pool(name=f"qf_pool{rep}", bufs=1) as qf_pool,
                    tc.tile_pool(name=f"kvres{rep}", bufs=1) as kvres,
                ):
                    qf = [
                        qf_pool.tile([P, TQ], BF16, name=f"qf{i}", tag=f"qf{i}")
                        for i in range(3 * NJ)
                    ]
                    ckv_full = kvres.tile([P, NKV, S], BF16, name="ckv_full")
                    kpe_sb = kvres.tile([P, S], BF16, name="kpe_sb")
                    zrow = kvres.tile([H, TQ], F32, name="zrow")
                    nc.vector.memset(zrow, 0.0)

                    with tc.tile_pool(name=f"qa_pool{rep}", bufs=1) as qa_pool:
                        qa_sb = qa_pool.tile([P, NKQ, TQ], BF16, name="qa_sb")
                        ib_sb = [
                            qa_pool.tile([P, NT], BF16, name=f"ibq{q}", tag=f"ibq{q}")
                            for q in range(NQT)
                        ]

                        with tc.tile_pool(name=f"xq_pool{rep}", bufs=1) as xkv:
                            xq_sb = xkv.tile([P, NKH, TQ], BF16, name="xq_sb")
                            for xc in range(4):
                                nc.sync.dma_start(
                                    xq_sb[:, 4 * xc : 4 * (xc + 1), :],
                                    xq[
                                        :, xc * 4 * TQ : (xc + 1) * 4 * TQ
                                    ].rearrange("p (k t) -> p k t", k=4),
                                )

                            # ====== P1: kv_a down-proj (own half) + exchange =
                            with (
                                tc.tile_pool(name=f"p1s{rep}", bufs=1) as p1s,
                                tc.tile_pool(name=f"p1ps{rep}", bufs=1, space="PSUM") as p1ps,
                            ):
                                wkva_sb = p1s.tile(
                                    [P, NKH, KVA_COLS], BF16, name="wkva_sb"
                                )
                                nc.sync.dma_start(wkva_sb, wkva[:, :, :])
                                ck = p1s.tile([D_ROPE, TQ], BF16, name="ck")
                                sk = p1s.tile([D_ROPE, TQ], BF16, name="sk")
                                nc.sync.dma_start(ck, cosk[:, :])
                                nc.sync.dma_start(sk, sink[:, :])
                                kpe_own = p1s.tile([P, TQ], BF16, name="kpe_own")
                                nc.vector.memset(kpe_own[D_ROPE:], 0.0)
                                ckv_own = p1s.tile([P, NKV, TQ], BF16, name="ckv_own")
                                for tt in range(NQT):
                                    tcs = slice(tt * NT, (tt + 1) * NT)
                                    acc_sq = p1s.tile(
                                        [P, NT], BF16, name="acc_sq",
                                        tag="acc_sq", bufs=2,
                                    )
                                    for m in [4, 0, 1, 2, 3]:
                                        pt = p1ps.tile(
                                            [P, NT], F32, name="pt", tag="kv_ps", bufs=3
                                        )
                                        for k in range(NKH):
                                            nc.tensor.matmul(
                                                pt,
                                                wkva_sb[:, k, m * P : (m + 1) * P],
                                                xq_sb[:, k, tcs],
                                                start=(k == 0),
                                                stop=(k == NKH - 1),
                                            )
                                        if m == 4:
                                            rot = p1s.tile(
                                                [D_ROPE, NT], BF16, name="rot",
                                                tag="krot", bufs=2,
                                            )
                                            nc.vector.tensor_scalar_mul(
                                                rot[0:32], pt[32:64], -1.0
                                            )
                                            nc.vector.tensor_copy(rot[32:64], pt[0:32])
                                            tmp = p1s.tile(
                                                [D_ROPE, NT], BF16, name="tmp",
                                                tag="ktmp", bufs=2,
                                            )
                                            nc.vector.tensor_mul(tmp, rot, sk[:, tcs])
                                            kc_sb = p1s.tile(
                                                [D_ROPE, NT], BF16, name="kc_sb",
                                                tag="kcos", bufs=2,
                                            )
                                            nc.vector.tensor_mul(
                                                kc_sb, pt[:D_ROPE], ck[:, tcs]
                                            )
                                            nc.vector.tensor_add(
                                                kpe_own[:D_ROPE, tcs], kc_sb, tmp
                                            )
                                        else:
                                            nc.vector.tensor_copy(
                                                ckv_own[:, m, tcs], pt
                                            )
                                            sq = p1s.tile(
                                                [P, NT], BF16, name="sq",
                                                tag="sq", bufs=3,
                                            )
                                            nc.scalar.square(sq, pt)
                                            if m == 0:
                                                nc.vector.tensor_copy(acc_sq, sq)
                                            else:
                                                nc.vector.tensor_add(
                                                    acc_sq, acc_sq, sq
                                                )
                                    part = p1ps.tile(
                                        [1, NT], F32, name="part", tag="part", bufs=2
                                    )
                                    nc.tensor.matmul(
                                        part, ones_col_bf, acc_sq, start=True, stop=True
                                    )
                                    rms = p1s.tile(
                                        [1, NT], F32, name="rms", tag="rms", bufs=2
                                    )
                                    nc.scalar.activation(
                                        rms, part, SQRT, bias=eps_t, scale=1.0 / KV_LORA
                                    )
                                    inv = p1s.tile(
                                        [1, NT], F32R, name="inv", tag="inv", bufs=2
                                    )
                                    with nc.allow_low_precision(
                                        reason="f32r is fp32-width"
                                    ):
                                        nc.vector.reciprocal(inv, rms)
                                    bc = p1ps.tile(
                                        [P, NT], F32, name="bc", tag="part", bufs=2
                                    )
                                    nc.tensor.matmul(
                                        bc, ones_row, inv, start=True, stop=True
                                    )
                                    ib = p1s.tile(
                                        [P, NT], BF16, name="ib", tag="ib", bufs=2
                                    )
                                    nc.vector.tensor_copy(ib, bc)
                                    for m in range(NKV):
                                        sl = ckv_own[:, m, tcs]
                                        nc.vector.tensor_mul(sl, sl, ib)
                                # exchange: compressed ckv + k_pe only (1.2 MB)
                                for m in range(NKV):
                                    nc.gpsimd.dma_start(ckx_loc[m], ckv_own[:, m, :])
                                nc.gpsimd.dma_start(ckx_loc[NKV], kpe_own)
                                nc.gpsimd.collective_compute(
                                    "AllGather",
                                    mybir.AluOpType.bypass,
                                    replica_groups=PAIRS,
                                    ins=[ckx_loc[:, :, :].opt()],
                                    outs=[ckx_gth[:, :, :, :].opt()],
                                )
                                for sl in range(2):
                                    scs = slice(sl * TQ, (sl + 1) * TQ)
                                    for k in range(NKV):
                                        nc.gpsimd.dma_start(
                                            ckv_full[:, k, scs], ckx_gth[sl, k]
                                        )
                                    for d in range(2):
                                        nc.gpsimd.dma_start(
                                            kpe_sb[d * D_ROPE : (d + 1) * D_ROPE, scs],
                                            ckx_gth[sl, NKV, :D_ROPE, :],
                                        )

                            # ============ P3a: q_a down-proj + RMS ===========
                            with (
                                tc.tile_pool(name=f"p3s{rep}", bufs=1) as p3s,
                                tc.tile_pool(name=f"p3ps{rep}", bufs=1, space="PSUM") as p3ps,
                            ):
                                acc_sq3 = [
                                    p3s.tile(
                                        [P, NT], BF16, name=f"accsq{q}", tag=f"accsq{q}"
                                    )
                                    for q in range(NQT)
                                ]
                                for m in range(NKQ):
                                    wqa_c = p3s.tile(
                                        [P, NKH * P], BF16, name="wqa_c",
                                        tag="wqa_c", bufs=3,
                                    )
                                    nc.sync.dma_start(wqa_c, wqa[:, m, :])
                                    pts = [
                                        p3ps.tile(
                                            [P, NT], F32, name="pt", tag="qa_ps", bufs=4
                                        )
                                        for _ in range(NQT)
                                    ]
                                    for k in range(NKH):
                                        for q in range(NQT):
                                            nc.tensor.matmul(
                                                pts[q],
                                                wqa_c[:, k * P : (k + 1) * P],
                                                xq_sb[:, k, q * NT : (q + 1) * NT],
                                                start=(k == 0),
                                                stop=(k == NKH - 1),
                                            )
                                    for q in range(NQT):
                                        nc.vector.tensor_copy(
                                            qa_sb[:, m, q * NT : (q + 1) * NT], pts[q]
                                        )
                                        sq = p3s.tile(
                                            [P, NT], BF16, name="sq", tag="sq", bufs=3
                                        )
                                        nc.scalar.square(sq, pts[q])
                                        if m == 0:
                                            nc.vector.tensor_copy(acc_sq3[q], sq)
                                        else:
                                            nc.vector.tensor_add(
                                                acc_sq3[q], acc_sq3[q], sq
                                            )
                                for q in range(NQT):
                                    qcs = slice(q * NT, (q + 1) * NT)
                                    part = p3ps.tile(
                                        [1, NT], F32, name="part", tag="part", bufs=2
                                    )
                                    nc.tensor.matmul(
                                        part, ones_col_bf, acc_sq3[q],
                                        start=True, stop=True,
                                    )
                                    rms = p3s.tile(
                                        [1, NT], F32, name="rms", tag="rms", bufs=2
                                    )
                                    nc.scalar.activation(
                                        rms, part, SQRT, bias=eps_t, scale=1.0 / Q_LORA
                                    )
                                    inv = p3s.tile(
                                        [1, NT], F32R, name="inv", tag="inv", bufs=2
                                    )
                                    with nc.allow_low_precision(
                                        reason="f32r is fp32-width"
                                    ):
                                        nc.vector.reciprocal(inv, rms)
                                    bc = p3ps.tile(
                                        [P, NT], F32, name="bc", tag="part", bufs=2
                                    )
                                    nc.tensor.matmul(
                                        bc, ones_row, inv, start=True, stop=True
                                    )
                                    # scaling is folded into the q_b output
                                    nc.vector.tensor_copy(ib_sb[q], bc)

                        # ============ P3b: q_b up-proj + rope ================
                        with (
                            tc.tile_pool(name=f"p4s{rep}", bufs=1) as p4s,
                            tc.tile_pool(name=f"p4ps{rep}", bufs=1, space="PSUM") as p4ps,
                        ):
                            cq = p4s.tile([P, TQ], BF16, name="cq")
                            sq_t = p4s.tile([P, TQ], BF16, name="sq_t")
                            nc.sync.dma_start(cq, cosq[:, :])
                            nc.sync.dma_start(sq_t, sinq[:, :])
                            for j in range(NJ):
                                wqb_c = p4s.tile(
                                    [P, NKQ * QBJ_COLS], BF16, name="wqb_c",
                                    tag="wqb_c", bufs=2,
                                )
                                nc.sync.dma_start(wqb_c, wqb[:, j, :])
                                for mi in range(3):
                                    pts = [
                                        p4ps.tile(
                                            [P, NT], F32, name="pt", tag="qf_ps", bufs=6
                                        )
                                        for _ in range(NQT)
                                    ]
                                    for k in range(NKQ):
                                        for q in range(NQT):
                                            nc.tensor.matmul(
                                                pts[q],
                                                wqb_c[
                                                    :,
                                                    k * QBJ_COLS
                                                    + mi * P : k * QBJ_COLS
                                                    + (mi + 1) * P,
                                                ],
                                                qa_sb[:, k, q * NT : (q + 1) * NT],
                                                start=(k == 0),
                                                stop=(k == NKQ - 1),
                                            )
                                    if mi < 2:
                                        for q in range(NQT):
                                            qcs = slice(q * NT, (q + 1) * NT)
                                            nc.vector.tensor_mul(
                                                qf[3 * j + mi][:, qcs],
                                                pts[q],
                                                ib_sb[q],
                                            )
                                    else:
                                        for q in range(NQT):
                                            qcs = slice(q * NT, (q + 1) * NT)
                                            pe_bf = p4s.tile(
                                                [P, NT], BF16, name="pe_bf",
                                                tag="pe_bf", bufs=2,
                                            )
                                            nc.vector.tensor_mul(
                                                pe_bf, pts[q], ib_sb[q]
                                            )
                                            rot = p4s.tile(
                                                [P, NT], BF16, name="rot",
                                                tag="rot", bufs=2,
                                            )
                                            nc.vector.tensor_scalar_mul(
                                                rot[0:32], pe_bf[32:64], -1.0
                                            )
                                            nc.vector.tensor_copy(
                                                rot[32:64], pe_bf[0:32]
                                            )
                                            nc.vector.tensor_scalar_mul(
                                                rot[64:96], pe_bf[96:128], -1.0
                                            )
                                            nc.vector.tensor_copy(
                                                rot[96:128], pe_bf[64:96]
                                            )
                                            tmp = p4s.tile(
                                                [P, NT], BF16, name="tmp",
                                                tag="rtmp", bufs=2,
                                            )
                                            nc.vector.tensor_mul(
                                                tmp, rot, sq_t[:, qcs]
                                            )
                                            dst = qf[3 * j + 2][:, qcs]
                                            nc.vector.tensor_mul(dst, pe_bf, cq[:, qcs])
                                            nc.vector.tensor_add(dst, dst, tmp)

                    # ====== P45: kv_b up-proj (full seq) + attention =========
                    with tc.tile_pool(name=f"p6w{rep}", bufs=1) as p6w:
                      with tc.tile_pool(name=f"p5s{rep}", bufs=1) as p5s:
                        ot_tiles = [
                            p5s.tile([P, TQ], BF16, name=f"ot{h}", tag=f"ot{h}")
                            for h in range(H)
                        ]
                        with tc.tile_pool(
                            name=f"p5ps{rep}", bufs=1, space="PSUM"
                        ) as p5ps:
                            for g in range(4):
                                wvbv_c = p5s.tile(
                                    [P, NKV, NT], BF16, name="wvbv_c",
                                    tag="wvbv_c", bufs=1,
                                )
                                nc.sync.dma_start(wvbv_c, wvb_v[:, g])
                                v_g = p5s.tile(
                                    [P, S // P, NT], BF16, name="v_g", tag="v_g",
                                    bufs=2,
                                )
                                for ti in range(S // P):
                                    pt = p5ps.tile(
                                        [P, NT], F32, name="pt", tag="mm", bufs=6
                                    )
                                    for k in range(NKV):
                                        nc.tensor.matmul(
                                            pt,
                                            ckv_full[:, k, ti * P : (ti + 1) * P],
                                            wvbv_c[:, k],
                                            start=(k == 0),
                                            stop=(k == NKV - 1),
                                        )
                                    nc.vector.tensor_copy(v_g[:, ti], pt)
                                for j in (2 * g, 2 * g + 1):
                                    h0, h1 = 2 * j, 2 * j + 1
                                    kn = {}
                                    for d, h in ((0, h0), (1, h1)):
                                        wvbk_c = p5s.tile(
                                            [P, NKV, P], BF16, name="wvbk_c",
                                            tag="wvbk_c", bufs=2,
                                        )
                                        nc.sync.dma_start(wvbk_c, wvb_k[:, h])
                                        knh = p5s.tile(
                                            [P, S], BF16, name="knh", tag="knh", bufs=2
                                        )
                                        for tt in range(S // NT):
                                            pt = p5ps.tile(
                                                [P, NT], F32, name="pt",
                                                tag="mm", bufs=6,
                                            )
                                            for k in range(NKV):
                                                nc.tensor.matmul(
                                                    pt,
                                                    wvbk_c[:, k],
                                                    ckv_full[
                                                        :, k, tt * NT : (tt + 1) * NT
                                                    ],
                                                    start=(k == 0),
                                                    stop=(k == NKV - 1),
                                                )
                                            nc.vector.tensor_copy(
                                                knh[:, tt * NT : (tt + 1) * NT], pt
                                            )
                                        kn[d] = knh
                                    qn = {0: qf[3 * j], 1: qf[3 * j + 1]}
                                    qpe = {
                                        d: qf[3 * j + 2][
                                            d * D_ROPE : (d + 1) * D_ROPE, :
                                        ]
                                        for d in range(2)
                                    }
                                    kpe = {
                                        d: kpe_sb[d * D_ROPE : (d + 1) * D_ROPE, :]
                                        for d in range(2)
                                    }
                                    for qt in range(NQT):
                                        qcs = slice(qt * NT, (qt + 1) * NT)
                                        pos = {}
                                        accs = {}
                                        for d in range(2):
                                            pos[d] = p5ps.tile(
                                                [P, NT], F32, name="po",
                                                tag="po", bufs=2,
                                            )
                                            accs[d] = p5s.tile(
                                                [P, NT], BF16, name="acc",
                                                tag="acc", bufs=2,
                                            )
                                        prev_p = None
                                        for kc in range(S // P):
                                            kcs = slice(kc * P, (kc + 1) * P)
                                            pst = {
                                                d: p5ps.tile(
                                                    [P, NT], F32, name="pst",
                                                    tag="mm", bufs=6,
                                                )
                                                for d in range(2)
                                            }
                                            for d in range(2):
                                                nc.tensor.matmul(
                                                    pst[d],
                                                    kn[d][:, kcs],
                                                    qn[d][:, qcs],
                                                    start=True,
                                                    stop=False,
                                                )
                                            # 64-row rope matmuls: d=0 on
                                            # partitions 0:64, d=1 on 64:128 →
                                            # disjoint row-groups, concurrent
                                            for d in range(2):
                                                nc.tensor.matmul(
                                                    pst[d],
                                                    kpe[d][:, kcs],
                                                    qpe[d][:, qcs],
                                                    start=False,
                                                    stop=True,
                                                )
                                            p_sbs = {}
                                            for d in range(2):
                                                p_sbs[d] = p5s.tile(
                                                    [P, NT], BF16, name="p_sb",
                                                    tag="p_sb", bufs=4,
                                                )
                                                nc.scalar.activation(
                                                    p_sbs[d], pst[d], EXP, scale=SCALE
                                                )
                                            for d in range(2):
                                                if kc == 0:
                                                    nc.vector.tensor_copy(
                                                        accs[d], p_sbs[d]
                                                    )
                                                else:
                                                    nc.vector.tensor_add(
                                                        accs[d], accs[d], p_sbs[d]
                                                    )
                                            # p*v lags one key-chunk so PE
                                            # never waits on the current exp
                                            if prev_p is not None:
                                                pk, pp = prev_p
                                                for d, hj in ((0, h0 % 4), (1, h1 % 4)):
                                                    nc.tensor.matmul(
                                                        pos[d],
                                                        v_g[:, pk, hj * P : (hj + 1) * P],
                                                        pp[d],
                                                        start=(pk == 0),
                                                        stop=False,
                                                        skip_group_check=True,
                                                    )
                                            prev_p = (kc, p_sbs)
                                        pk, pp = prev_p
                                        for d, hj in ((0, h0 % 4), (1, h1 % 4)):
                                            nc.tensor.matmul(
                                                pos[d],
                                                v_g[:, pk, hj * P : (hj + 1) * P],
                                                pp[d],
                                                start=False,
                                                stop=True,
                                                skip_group_check=True,
                                            )
                                        for d, h in ((0, h0), (1, h1)):
                                            zsel = p5ps.tile(
                                                [H, NT], F32, name="zsel",
                                                tag="mm", bufs=6,
                                            )
                                            nc.tensor.matmul(
                                                zsel,
                                                selcol[:, h * H : (h + 1) * H],
                                                accs[d],
                                                start=True,
                                                stop=True,
                                            )
                                            nc.vector.tensor_add(
                                                zrow[:, qcs], zrow[:, qcs], zsel
                                            )
                                            nc.vector.tensor_copy(
                                                ot_tiles[h][:, qcs], pos[d]
                                            )
                            zinv = p5s.tile([H, TQ], F32R, name="zinv")
                            with nc.allow_low_precision(reason="f32r is fp32-width"):
                                nc.vector.reciprocal(zinv, zrow)

                        # -------- deferred softmax normalization ------------
                        with tc.tile_pool(
                            name=f"nrm{rep}", bufs=1, space="PSUM"
                        ) as nrmps:
                            for h in range(H):
                                for qt in range(NQT):
                                    qcs = slice(qt * NT, (qt + 1) * NT)
                                    bc = nrmps.tile(
                                        [P, NT], F32, name="bc", tag="bc", bufs=4
                                    )
                                    nc.tensor.matmul(
                                        bc,
                                        selbrd[:, h * P : (h + 1) * P],
                                        zinv[:, qcs],
                                        start=True,
                                        stop=True,
                                    )
                                    nc.vector.tensor_mul(
                                        ot_tiles[h][:, qcs], ot_tiles[h][:, qcs], bc
                                    )

                        # ================= P6: output projection =============
                        with tc.tile_pool(
                            name=f"p6ps{rep}", bufs=1, space="PSUM"
                        ) as p6ps:
                            for wq in range(4):
                                wo_c = p6w.tile(
                                    [P, NKO * NT], BF16, name="wo_c",
                                    tag="wo_c", bufs=2,
                                )
                                nc.sync.dma_start(wo_c, wo[:, wq, :])
                                pts = {}
                                for mi in range(4):
                                    for q in range(NQT):
                                        pts[(mi, q)] = p6ps.tile(
                                            [P, NT], F32, name="pt",
                                            tag=f"oo{mi}{q}", bufs=1,
                                        )
                                for k in range(NKO):
                                    for mi in range(4):
                                        for q in range(NQT):
                                            nc.tensor.matmul(
                                                pts[(mi, q)],
                                                wo_c[
                                                    :,
                                                    k * NT + mi * P : k * NT
                                                    + (mi + 1) * P,
                                                ],
                                                ot_tiles[k][:, q * NT : (q + 1) * NT],
                                                start=(k == 0),
                                                stop=(k == NKO - 1),
                                            )
                                for mi in range(4):
                                    m = 4 * wq + mi
                                    for q in range(NQT):
                                        sb = p5s.tile(
                                            [P, NT], F32, name="sb", tag="o_sb", bufs=2
                                        )
                                        nc.scalar.activation(sb, pts[(mi, q)], COPY)
                                        nc.sync.dma_start(
                                            outT[
                                                m * P : (m + 1) * P,
                                                q * NT : (q + 1) * NT,
                                            ],
                                            sb,
                                        )

    nc.compile()
    _CACHE[key] = nc
    return nc


# ======================= host-side preparation ===========================


def _deint_perm(d):
    half = d // 2
    perm = np.empty(d, dtype=np.int64)
    perm[:half] = 2 * np.arange(half)
    perm[half:] = 2 * np.arange(half) + 1
    return perm


def _kpanel(w_t):
    """[K, M] -> [P, K//P * M] k-major panels: out[p, k*M + c] = w_t[k*P+p, c]."""
    K, M = w_t.shape
    return np.ascontiguousarray(
        w_t.reshape(K // P, P, M).transpose(1, 0, 2).reshape(P, -1)
    )


def prepare_host_inputs(x, q_a_w, q_a_ln_w, q_b_w, kv_a_w, kv_a_ln_w, kv_b_w, o_w):
    import ml_dtypes

    bf16 = ml_dtypes.bfloat16
    perm = _deint_perm(D_ROPE)

    # ---- q_b: ln folded, per-j blocks [nope0|nope1|pe0:pe1], j-major chunks
    qb = (q_b_w * q_a_ln_w[None, :]).reshape(H, D_QK, Q_LORA)
    qb_rows = np.zeros((NJ, QBJ_COLS, Q_LORA), dtype=np.float32)
    for j in range(NJ):
        h0, h1 = 2 * j, 2 * j + 1
        qb_rows[j, 0:P] = qb[h0, :D_NOPE]
        qb_rows[j, P : 2 * P] = qb[h1, :D_NOPE]
        qb_rows[j, 2 * P : 2 * P + D_ROPE] = qb[h0, D_NOPE:][perm]
        qb_rows[j, 2 * P + D_ROPE : 3 * P] = qb[h1, D_NOPE:][perm]
    wqb_host = np.ascontiguousarray(
        qb_rows.reshape(NJ, QBJ_COLS, NKQ, P).transpose(3, 0, 2, 1).reshape(P, NJ, -1)
    ).astype(bf16)

    # ---- q_a: m-major chunks: wqa[p, m, k*P + c] = q_a_w[m*P+c, k*P+p]
    wqa_host = np.ascontiguousarray(
        q_a_w.reshape(NKQ, P, NKH, P).transpose(3, 0, 2, 1).reshape(P, NKQ, -1)
    ).astype(bf16)

    # ---- kv_a: aug rows [512 ckv | pe(64) | pad(64)], k-major panels
    kva_aug = np.zeros((KVA_COLS, HID), dtype=np.float32)
    kva_aug[:KV_LORA] = kv_a_w[:KV_LORA]
    kva_aug[KV_LORA : KV_LORA + D_ROPE] = kv_a_w[KV_LORA:][perm]
    wkva_host = (
        _kpanel(np.ascontiguousarray(kva_aug.T)).reshape(P, NKH, KVA_COLS).astype(bf16)
    )

    # ---- kv_b: ln folded; split kn / v with chunked layouts
    kvb = (kv_b_w * kv_a_ln_w[None, :]).reshape(H, D_NOPE + D_V, KV_LORA)
    kn_w = kvb[:, :D_NOPE].reshape(H * D_NOPE, KV_LORA).T  # [KV_LORA, H*P]
    v_w = kvb[:, D_NOPE:].reshape(H * D_V, KV_LORA).T  # [KV_LORA, H*D_V]
    # wvb_v[p, g, k, c] = v_w[k*P+p, g*NT+c]
    wvb_v_host = np.ascontiguousarray(
        v_w.reshape(NKV, P, 4, NT).transpose(1, 2, 0, 3)
    ).astype(bf16)
    # wvb_k[p, h, k, c] = kn_w[k*P+p, h*P+c]
    wvb_k_host = np.ascontiguousarray(
        kn_w.reshape(NKV, P, H, P).transpose(1, 2, 0, 3)
    ).astype(bf16)

    # ---- o_w: quarter-major: wo[p, wq, k*NT + mi*P + c] = o_w.T[k*P+p, ...]
    owt = np.ascontiguousarray(o_w.T)  # [H*D_V, HID]
    wo_host = np.ascontiguousarray(
        owt.reshape(NKO, P, 4, 4 * P).transpose(1, 2, 0, 3).reshape(P, 4, -1)
    ).astype(bf16)

    inv_freq = 1.0 / (THETA ** (np.arange(0, D_ROPE, 2, dtype=np.float32) / D_ROPE))
    t = np.arange(S, dtype=np.float32)
    ang = np.outer(inv_freq, t)
    cos = np.concatenate([np.cos(ang)] * 2, axis=0).astype(np.float32)  # [64, S]
    sin = np.concatenate([np.sin(ang)] * 2, axis=0).astype(np.float32)
    cos2 = np.concatenate([cos, cos], axis=0)  # [128, S]
    sin2 = np.concatenate([sin, sin], axis=0)

    selcol_host = np.zeros((P, H * H), dtype=np.float32)
    for h in range(H):
        selcol_host[:, h * H + h] = 1.0
    selbrd_host = np.zeros((H, H * P), dtype=np.float32)
    for h in range(H):
        selbrd_host[h, h * P : (h + 1) * P] = 1.0

    shared = {
        "ones": np.ones((P, 1), dtype=np.float32),
        "selcol": selcol_host.astype(bf16),
        "selbrd": selbrd_host,
        "wqa": wqa_host,
        "wqb": wqb_host,
        "wkva": wkva_host,
        "wvb_v": wvb_v_host,
        "wvb_k": wvb_k_host,
        "wo": wo_host,
    }
    per_core = []
    for c in range(N_CORES):
        b, qh = c // 2, c % 2
        xTb = np.ascontiguousarray(x[b].T)  # [HID, S]
        qs = qh * TQ
        xs = xTb[:, qs : qs + TQ]  # [HID, TQ]
        m = dict(shared)
        m["xq"] = np.ascontiguousarray(
            xs.reshape(NKH, P, TQ).transpose(1, 0, 2).reshape(P, -1)
        ).astype(bf16)
        m["cosq"] = np.ascontiguousarray(cos2[:, qs : qs + TQ]).astype(bf16)
        m["sinq"] = np.ascontiguousarray(sin2[:, qs : qs + TQ]).astype(bf16)
        m["cosk"] = np.ascontiguousarray(cos[:, qs : qs + TQ]).astype(bf16)
        m["sink"] = np.ascontiguousarray(sin[:, qs : qs + TQ]).astype(bf16)
        per_core.append(m)
    return per_core


def kernel(x, q_a_w, q_a_ln_w, q_b_w, kv_a_w, kv_a_ln_w, kv_b_w, o_w):
    from concourse.bass_utils import run_bass_kernel_spmd

    nc = build_nc()
    per_core = prepare_host_inputs(
        np.asarray(x),
        np.asarray(q_a_w),
        np.asarray(q_a_ln_w),
        np.asarray(q_b_w),
        np.asarray(kv_a_w),
        np.asarray(kv_a_ln_w),
        np.asarray(kv_b_w),
        np.asarray(o_w),
    )
    res = run_bass_kernel_spmd(nc, per_core, core_ids=list(range(N_CORES)))
    out = np.empty((B, S, HID), dtype=np.float32)
    for c in range(N_CORES):
        b, qh = c // 2, c % 2
        out[b, qh * TQ : (qh + 1) * TQ] = res.results[c]["outT"].T
    return out


# revision 28
# speedup vs baseline: 1.1364x; 1.1364x over previous
"""DeepSeek-V3 MLA attention wrapper kernel for 8 Trainium2 NeuronCores.

Sharding: core c = (b, qh) with b = c // 2 (batch index), qh = c % 2 (token
half). Each core computes the compressed KV (kv_a down-proj + rope'd k_pe)
for its OWN 1024-token half, then the two cores of a batch exchange just the
compressed c_kv/k_pe (1.2 MB) with a 2-core AllGather. Each core then runs
the kv_b up-projection for the FULL sequence locally, keeping k_nope/v
entirely in SBUF (never spilled to HBM). Q path + attention + output
projection cover the core's own 1024 query rows; per-core outputs are
disjoint row blocks the host concats.

Key design points:
- All weights/activations bf16 (fp32 PSUM accumulate); panelized weight
  layouts so weights stream as a few large contiguous DMAs on sync.
- Only the tiny AllGather and its consumers live on gpsimd, so weight
  streaming never queues behind a collective wait.
- Attention processes head pairs (2j, 2j+1): their 64-row rope matmuls
  go to disjoint PE row-groups (partitions 0:64 / 64:128) and run
  concurrently in the array.
- Softmax row-sums: a selector-matrix matmul drops each head's key-sum into
  row h of a [16, TQ] accumulator; one batched DVE reciprocal; the 1/Z
  broadcast is another selector matmul. Normalization is deferred to after
  the attention loop (p*v stays unnormalized in PSUM/bf16 meanwhile).
"""

import sys

sys.path.insert(0, "/opt/trn_rl_repo")

import numpy as np

import concourse.tile as tile
from concourse import bacc, mybir

B, S, H = 4, 2048, 16
HID = 2048
Q_LORA = 1536
KV_LORA = 512
D_NOPE, D_ROPE, D_V = 128, 64, 128
D_QK = D_NOPE + D_ROPE  # 192
THETA = 10000.0
EPS = 1e-6

P = 128
TQ = S // 2  # tokens per core (queries); own KV half
NT = 512  # matmul free-dim tile
NQT = TQ // NT  # 2
N_CORES = 8
PAIRS = [[0, 1], [2, 3], [4, 5], [6, 7]]

NKH = HID // P  # 16
NKQ = Q_LORA // P  # 12
NKV = KV_LORA // P  # 4
NKO = H * D_V // P  # 16
NJ = H // 2  # 8
KVA_COLS = KV_LORA + P  # 512 ckv | pe(64)+pad(64)
QBJ_COLS = 3 * P  # per-j: nope0 | nope1 | [pe0|pe1]

F32 = mybir.dt.float32
F32R = mybir.dt.float32r
BF16 = mybir.dt.bfloat16
EXP = mybir.ActivationFunctionType.Exp
SQRT = mybir.ActivationFunctionType.Sqrt
COPY = mybir.ActivationFunctionType.Copy

SCALE = float(D_QK) ** -0.5

_CACHE = {}


def build_nc():
    import os

    REP = int(os.environ.get("KREPEAT", "1"))
    key = ("nc", REP)
    if key in _CACHE:
        return _CACHE[key]
    nc = bacc.Bacc(None, target_bir_lowering=False, num_devices=N_CORES)

    xq = nc.dram_tensor("xq", [P, NKH * TQ], BF16, kind="ExternalInput")
    wkva = nc.dram_tensor("wkva", [P, NKH, KVA_COLS], BF16, kind="ExternalInput")
    wvb_v = nc.dram_tensor("wvb_v", [P, 4, NKV, NT], BF16, kind="ExternalInput")
    wvb_k = nc.dram_tensor("wvb_k", [P, H, NKV, P], BF16, kind="ExternalInput")
    wqa = nc.dram_tensor("wqa", [P, NKQ, NKH * P], BF16, kind="ExternalInput")
    wqb = nc.dram_tensor("wqb", [P, NJ, NKQ * QBJ_COLS], BF16, kind="ExternalInput")
    wo = nc.dram_tensor("wo", [P, 4, NKO * NT], BF16, kind="ExternalInput")
    cosq = nc.dram_tensor("cosq", [P, TQ], BF16, kind="ExternalInput")
    sinq = nc.dram_tensor("sinq", [P, TQ], BF16, kind="ExternalInput")
    cosk = nc.dram_tensor("cosk", [D_ROPE, TQ], BF16, kind="ExternalInput")
    sink = nc.dram_tensor("sink", [D_ROPE, TQ], BF16, kind="ExternalInput")
    ones_in = nc.dram_tensor("ones", [P, 1], F32R, kind="ExternalInput")
    selcol_in = nc.dram_tensor("selcol", [P, H * H], BF16, kind="ExternalInput")
    selbrd_in = nc.dram_tensor("selbrd", [H, H * P], F32R, kind="ExternalInput")
    outT = nc.dram_tensor("outT", [HID, TQ], F32, kind="ExternalOutput")

    # compressed-KV exchange buffers (HBM): 4 ckv slabs + 1 k_pe slab
    ckx_loc = nc.dram_tensor("ckx_loc", [NKV + 1, P, TQ], BF16, kind="Internal")
    ckx_gth = nc.dram_tensor("ckx_gth", [2, NKV + 1, P, TQ], BF16, kind="Internal")

    with tile.TileContext(nc) as tc:
        with tc.tile_pool(name="const", bufs=1) as const:
            ones_row = const.tile([1, P], F32R, name="ones_row")
            nc.sync.dma_start(ones_row, ones_in[:, :].rearrange("p one -> one p"))
            ones_col_bf = const.tile([P, 1], BF16, name="ones_col_bf")
            nc.vector.memset(ones_col_bf, 1.0)
            eps_t = const.tile([1, 1], F32, name="eps_t")
            nc.vector.memset(eps_t, EPS)
            selcol = const.tile([P, H * H], BF16, name="selcol")
            nc.sync.dma_start(selcol, selcol_in[:, :])
            selbrd = const.tile([H, H * P], F32R, name="selbrd")
            nc.sync.dma_start(selbrd, selbrd_in[:, :])

            for rep in range(REP):
                with (
                    tc.tile_pool(name=f"qf_pool{rep}", bufs=1) as qf_pool,
                    tc.tile_pool(name=f"kvres{rep}", bufs=1) as kvres,
                ):
                    qf = [
                        qf_pool.tile([P, TQ], BF16, name=f"qf{i}", tag=f"qf{i}")
                        for i in range(3 * NJ)
                    ]
                    ckv_full = kvres.tile([P, NKV, S], BF16, name="ckv_full")
                    kpe_sb = kvres.tile([P, S], BF16, name="kpe_sb")
                    zrow = kvres.tile([H, TQ], F32, name="zrow")
                    nc.vector.memset(zrow, 0.0)

                    with tc.tile_pool(name=f"qa_pool{rep}", bufs=1) as qa_pool:
                        qa_sb = qa_pool.tile([P, NKQ, TQ], BF16, name="qa_sb")
                        ib_sb = [
                            qa_pool.tile([P, NT], BF16, name=f"ibq{q}", tag=f"ibq{q}")
                            for q in range(NQT)
                        ]

                        with tc.tile_pool(name=f"xq_pool{rep}", bufs=1) as xkv:
                            xq_sb = xkv.tile([P, NKH, TQ], BF16, name="xq_sb")
                            for xc in range(4):
                                nc.sync.dma_start(
                                    xq_sb[:, 4 * xc : 4 * (xc + 1), :],
                                    xq[
                                        :, xc * 4 * TQ : (xc + 1) * 4 * TQ
                                    ].rearrange("p (k t) -> p k t", k=4),
                                )

                            # ====== P1: kv_a down-proj (own half) + exchange =
                            with (
                                tc.tile_pool(name=f"p1s{rep}", bufs=1) as p1s,
                                tc.tile_pool(name=f"p1ps{rep}", bufs=1, space="PSUM") as p1ps,
                            ):
                                wkva_sb = p1s.tile(
                                    [P, NKH, KVA_COLS], BF16, name="wkva_sb"
                                )
                                nc.sync.dma_start(wkva_sb, wkva[:, :, :])
                                ck = p1s.tile([D_ROPE, TQ], BF16, name="ck")
                                sk = p1s.tile([D_ROPE, TQ], BF16, name="sk")
                                nc.sync.dma_start(ck, cosk[:, :])
                                nc.sync.dma_start(sk, sink[:, :])
                                kpe_own = p1s.tile([P, TQ], BF16, name="kpe_own")
                                nc.vector.memset(kpe_own[D_ROPE:], 0.0)
                                ckv_own = p1s.tile([P, NKV, TQ], BF16, name="ckv_own")
                                for tt in range(NQT):
                                    tcs = slice(tt * NT, (tt + 1) * NT)
                                    acc_sq = p1s.tile(
                                        [P, NT], BF16, name="acc_sq",
                                        tag="acc_sq", bufs=2,
                                    )
                                    for m in [4, 0, 1, 2, 3]:
                                        pt = p1ps.tile(
                                            [P, NT], F32, name="pt", tag="kv_ps", bufs=3
                                        )
                                        for k in range(NKH):
                                            nc.tensor.matmul(
                                                pt,
                                                wkva_sb[:, k, m * P : (m + 1) * P],
                                                xq_sb[:, k, tcs],
                                                start=(k == 0),
                                                stop=(k == NKH - 1),
                                            )
                                        if m == 4:
                                            rot = p1s.tile(
                                                [D_ROPE, NT], BF16, name="rot",
                                                tag="krot", bufs=2,
                                            )
                                            nc.vector.tensor_scalar_mul(
                                                rot[0:32], pt[32:64], -1.0
                                            )
                                            nc.vector.tensor_copy(rot[32:64], pt[0:32])
                                            tmp = p1s.tile(
                                                [D_ROPE, NT], BF16, name="tmp",
                                                tag="ktmp", bufs=2,
                                            )
                                            nc.vector.tensor_mul(tmp, rot, sk[:, tcs])
                                            kc_sb = p1s.tile(
                                                [D_ROPE, NT], BF16, name="kc_sb",
                                                tag="kcos", bufs=2,
                                            )
                                            nc.vector.tensor_mul(
                                                kc_sb, pt[:D_ROPE], ck[:, tcs]
                                            )
                                            nc.vector.tensor_add(
                                                kpe_own[:D_ROPE, tcs], kc_sb, tmp
                                            )
                                        else:
                                            nc.vector.tensor_copy(
                                                ckv_own[:, m, tcs], pt
                                            )
                                            sq = p1s.tile(
                                                [P, NT], BF16, name="sq",
                                                tag="sq", bufs=3,
                                            )
                                            nc.scalar.square(sq, pt)
                                            if m == 0:
                                                nc.vector.tensor_copy(acc_sq, sq)
                                            else:
                                                nc.vector.tensor_add(
                                                    acc_sq, acc_sq, sq
                                                )
                                    part = p1ps.tile(
                                        [1, NT], F32, name="part", tag="part", bufs=2
                                    )
                                    nc.tensor.matmul(
                                        part, ones_col_bf, acc_sq, start=True, stop=True
                                    )
                                    rms = p1s.tile(
                                        [1, NT], F32, name="rms", tag="rms", bufs=2
                                    )
                                    nc.scalar.activation(
                                        rms, part, SQRT, bias=eps_t, scale=1.0 / KV_LORA
                                    )
                                    inv = p1s.tile(
                                        [1, NT], F32R, name="inv", tag="inv", bufs=2
                                    )
                                    with nc.allow_low_precision(
                                        reason="f32r is fp32-width"
                                    ):
                                        nc.vector.reciprocal(inv, rms)
                                    bc = p1ps.tile(
                                        [P, NT], F32, name="bc", tag="part", bufs=2
                                    )
                                    nc.tensor.matmul(
                                        bc, ones_row, inv, start=True, stop=True
                                    )
                                    ib = p1s.tile(
                                        [P, NT], BF16, name="ib", tag="ib", bufs=2
                                    )
                                    nc.vector.tensor_copy(ib, bc)
                                    for m in range(NKV):
                                        sl = ckv_own[:, m, tcs]
                                        nc.vector.tensor_mul(sl, sl, ib)
                                # exchange: compressed ckv + k_pe only (1.2 MB)
                                for m in range(NKV):
                                    nc.gpsimd.dma_start(ckx_loc[m], ckv_own[:, m, :])
                                nc.gpsimd.dma_start(ckx_loc[NKV], kpe_own)
                                nc.gpsimd.collective_compute(
                                    "AllGather",
                                    mybir.AluOpType.bypass,
                                    replica_groups=PAIRS,
                                    ins=[ckx_loc[:, :, :].opt()],
                                    outs=[ckx_gth[:, :, :, :].opt()],
                                )
                                for sl in range(2):
                                    scs = slice(sl * TQ, (sl + 1) * TQ)
                                    for k in range(NKV):
                                        nc.gpsimd.dma_start(
                                            ckv_full[:, k, scs], ckx_gth[sl, k]
                                        )
                                    for d in range(2):
                                        nc.gpsimd.dma_start(
                                            kpe_sb[d * D_ROPE : (d + 1) * D_ROPE, scs],
                                            ckx_gth[sl, NKV, :D_ROPE, :],
                                        )

                            # ============ P3a: q_a down-proj + RMS ===========
                            with (
                                tc.tile_pool(name=f"p3s{rep}", bufs=1) as p3s,
                                tc.tile_pool(name=f"p3ps{rep}", bufs=1, space="PSUM") as p3ps,
                            ):
                                acc_sq3 = [
                                    p3s.tile(
                                        [P, NT], BF16, name=f"accsq{q}", tag=f"accsq{q}"
                                    )
                                    for q in range(NQT)
                                ]
                                for m in range(NKQ):
                                    wqa_c = p3s.tile(
                                        [P, NKH * P], BF16, name="wqa_c",
                                        tag="wqa_c", bufs=3,
                                    )
                                    nc.sync.dma_start(wqa_c, wqa[:, m, :])
                                    pts = [
                                        p3ps.tile(
                                            [P, NT], F32, name="pt", tag="qa_ps", bufs=4
                                        )
                                        for _ in range(NQT)
                                    ]
                                    for k in range(NKH):
                                        for q in range(NQT):
                                            nc.tensor.matmul(
                                                pts[q],
                                                wqa_c[:, k * P : (k + 1) * P],
                                                xq_sb[:, k, q * NT : (q + 1) * NT],
                                                start=(k == 0),
                                                stop=(k == NKH - 1),
                                            )
                                    for q in range(NQT):
                                        nc.vector.tensor_copy(
                                            qa_sb[:, m, q * NT : (q + 1) * NT], pts[q]
                                        )
                                        sq = p3s.tile(
                                            [P, NT], BF16, name="sq", tag="sq", bufs=3
                                        )
                                        nc.scalar.square(sq, pts[q])
                                        if m == 0:
                                            nc.vector.tensor_copy(acc_sq3[q], sq)
                                        else:
                                            nc.vector.tensor_add(
                                                acc_sq3[q], acc_sq3[q], sq
                                            )
                                for q in range(NQT):
                                    qcs = slice(q * NT, (q + 1) * NT)
                                    part = p3ps.tile(
                                        [1, NT], F32, name="part", tag="part", bufs=2
                                    )
                                    nc.tensor.matmul(
                                        part, ones_col_bf, acc_sq3[q],
                                        start=True, stop=True,
                                    )
                                    rms = p3s.tile(
                                        [1, NT], F32, name="rms", tag="rms", bufs=2
                                    )
                                    nc.scalar.activation(
                                        rms, part, SQRT, bias=eps_t, scale=1.0 / Q_LORA
                                    )
                                    inv = p3s.tile(
                                        [1, NT], F32R, name="inv", tag="inv", bufs=2
                                    )
                                    with nc.allow_low_precision(
                                        reason="f32r is fp32-width"
                                    ):
                                        nc.vector.reciprocal(inv, rms)
                                    bc = p3ps.tile(
                                        [P, NT], F32, name="bc", tag="part", bufs=2
                                    )
                                    nc.tensor.matmul(
                                        bc, ones_row, inv, start=True, stop=True
                                    )
                                    # scaling is folded into the q_b output
                                    nc.vector.tensor_copy(ib_sb[q], bc)

                        # ============ P3b: q_b up-proj + rope ================
                        with (
                            tc.tile_pool(name=f"p4s{rep}", bufs=1) as p4s,
                            tc.tile_pool(name=f"p4ps{rep}", bufs=1, space="PSUM") as p4ps,
                        ):
                            cq = p4s.tile([P, TQ], BF16, name="cq")
                            sq_t = p4s.tile([P, TQ], BF16, name="sq_t")
                            nc.sync.dma_start(cq, cosq[:, :])
                            nc.sync.dma_start(sq_t, sinq[:, :])
                            for j in range(NJ):
                                wqb_c = p4s.tile(
                                    [P, NKQ * QBJ_COLS], BF16, name="wqb_c",
                                    tag="wqb_c", bufs=2,
                                )
                                nc.sync.dma_start(wqb_c, wqb[:, j, :])
                                for mi in range(3):
                                    pts = [
                                        p4ps.tile(
                                            [P, NT], F32, name="pt", tag="qf_ps", bufs=6
                                        )
                                        for _ in range(NQT)
                                    ]
                                    for k in range(NKQ):
                                        for q in range(NQT):
                                            nc.tensor.matmul(
                                                pts[q],
                                                wqb_c[
                                                    :,
                                                    k * QBJ_COLS
                                                    + mi * P : k * QBJ_COLS
                                                    + (mi + 1) * P,
                                                ],
                                                qa_sb[:, k, q * NT : (q + 1) * NT],
                                                start=(k == 0),
                                                stop=(k == NKQ - 1),
                                            )
                                    if mi < 2:
                                        for q in range(NQT):
                                            qcs = slice(q * NT, (q + 1) * NT)
                                            nc.vector.tensor_mul(
                                                qf[3 * j + mi][:, qcs],
                                                pts[q],
                                                ib_sb[q],
                                            )
                                    else:
                                        for q in range(NQT):
                                            qcs = slice(q * NT, (q + 1) * NT)
                                            pe_bf = p4s.tile(
                                                [P, NT], BF16, name="pe_bf",
                                                tag="pe_bf", bufs=2,
                                            )
                                            nc.vector.tensor_mul(
                                                pe_bf, pts[q], ib_sb[q]
                                            )
                                            rot = p4s.tile(
                                                [P, NT], BF16, name="rot",
                                                tag="rot", bufs=2,
                                            )
                                            nc.vector.tensor_scalar_mul(
                                                rot[0:32], pe_bf[32:64], -1.0
                                            )
                                            nc.vector.tensor_copy(
                                                rot[32:64], pe_bf[0:32]
                                            )
                                            nc.vector.tensor_scalar_mul(
                                                rot[64:96], pe_bf[96:128], -1.0
                                            )
                                            nc.vector.tensor_copy(
                                                rot[96:128], pe_bf[64:96]
                                            )
                                            tmp = p4s.tile(
                                                [P, NT], BF16, name="tmp",
                                                tag="rtmp", bufs=2,
                                            )
                                            nc.vector.tensor_mul(
                                                tmp, rot, sq_t[:, qcs]
                                            )
                                            dst = qf[3 * j + 2][:, qcs]
                                            nc.vector.tensor_mul(dst, pe_bf, cq[:, qcs])
                                            nc.vector.tensor_add(dst, dst, tmp)

                    # ====== P45: kv_b up-proj (full seq) + attention =========
                    with tc.tile_pool(name=f"p6w{rep}", bufs=1) as p6w:
                      with tc.tile_pool(name=f"p5s{rep}", bufs=1) as p5s:
                        ot_tiles = [
                            p5s.tile([P, TQ], BF16, name=f"ot{h}", tag=f"ot{h}")
                            for h in range(H)
                        ]
                        with tc.tile_pool(
                            name=f"p5ps{rep}", bufs=1, space="PSUM"
                        ) as p5ps:
                            for g in range(4):
                                wvbv_c = p5s.tile(
                                    [P, NKV, NT], BF16, name="wvbv_c",
                                    tag="wvbv_c", bufs=1,
                                )
                                nc.sync.dma_start(wvbv_c, wvb_v[:, g])
                                v_g = p5s.tile(
                                    [P, S // P, NT], BF16, name="v_g", tag="v_g",
                                    bufs=2,
                                )
                                for ti in range(S // P):
                                    pt = p5ps.tile(
                                        [P, NT], F32, name="pt", tag="mm", bufs=6
                                    )
                                    for k in range(NKV):
                                        nc.tensor.matmul(
                                            pt,
                                            ckv_full[:, k, ti * P : (ti + 1) * P],
                                            wvbv_c[:, k],
                                            start=(k == 0),
                                            stop=(k == NKV - 1),
                                        )
                                    nc.vector.tensor_copy(v_g[:, ti], pt)
                                for j in (2 * g, 2 * g + 1):
                                    h0, h1 = 2 * j, 2 * j + 1
                                    kn = {}
                                    for d, h in ((0, h0), (1, h1)):
                                        wvbk_c = p5s.tile(
                                            [P, NKV, P], BF16, name="wvbk_c",
                                            tag="wvbk_c", bufs=2,
                                        )
                                        nc.sync.dma_start(wvbk_c, wvb_k[:, h])
                                        knh = p5s.tile(
                                            [P, S], BF16, name="knh", tag="knh", bufs=2
                                        )
                                        for tt in range(S // NT):
                                            pt = p5ps.tile(
                                                [P, NT], F32, name="pt",
                                                tag="mm", bufs=6,
                                            )
                                            for k in range(NKV):
                                                nc.tensor.matmul(
                                                    pt,
                                                    wvbk_c[:, k],
                                                    ckv_full[
                                                        :, k, tt * NT : (tt + 1) * NT
                                                    ],
                                                    start=(k == 0),
                                                    stop=(k == NKV - 1),
                                                )
                                            nc.vector.tensor_copy(
                                                knh[:, tt * NT : (tt + 1) * NT], pt
                                            )
                                        kn[d] = knh
                                    qn = {0: qf[3 * j], 1: qf[3 * j + 1]}
                                    qpe = {
                                        d: qf[3 * j + 2][
                                            d * D_ROPE : (d + 1) * D_ROPE, :
                                        ]
                                        for d in range(2)
                                    }
                                    kpe = {
                                        d: kpe_sb[d * D_ROPE : (d + 1) * D_ROPE, :]
                                        for d in range(2)
                                    }
                                    for qt in range(NQT):
                                        qcs = slice(qt * NT, (qt + 1) * NT)
                                        pos = {}
                                        accs = {}
                                        for d in range(2):
                                            pos[d] = p5ps.tile(
                                                [P, NT], F32, name="po",
                                                tag="po", bufs=2,
                                            )
                                            accs[d] = p5s.tile(
                                                [P, NT], BF16, name="acc",
                                                tag="acc", bufs=2,
                                            )
                                        prev_p = None
                                        for kc in range(S // P):
                                            kcs = slice(kc * P, (kc + 1) * P)
                                            pst = {
                                                d: p5ps.tile(
                                                    [P, NT], F32, name="pst",
                                                    tag="mm", bufs=6,
                                                )
                                                for d in range(2)
                                            }
                                            for d in range(2):
                                                nc.tensor.matmul(
                                                    pst[d],
                                                    kn[d][:, kcs],
                                                    qn[d][:, qcs],
                                                    start=True,
                                                    stop=False,
                                                )
                                            # 64-row rope matmuls: d=0 on
                                            # partitions 0:64, d=1 on 64:128 →
                                            # disjoint row-groups, concurrent
                                            for d in range(2):
                                                nc.tensor.matmul(
                                                    pst[d],
                                                    kpe[d][:, kcs],
                                                    qpe[d][:, qcs],
                                                    start=False,
                                                    stop=True,
                                                )
                                            p_sbs = {}
                                            for d in range(2):
                                                p_sbs[d] = p5s.tile(
                                                    [P, NT], BF16, name="p_sb",
                                                    tag="p_sb", bufs=4,
                                                )
                                                nc.scalar.activation(
                                                    p_sbs[d], pst[d], EXP, scale=SCALE
                                                )
                                            for d in range(2):
                                                if kc == 0:
                                                    nc.vector.tensor_copy(
                                                        accs[d], p_sbs[d]
                                                    )
                                                else:
                                                    nc.vector.tensor_add(
                                                        accs[d], accs[d], p_sbs[d]
                                                    )
                                            # p*v lags one key-chunk so PE
                                            # never waits on the current exp
                                            if prev_p is not None:
                                                pk, pp = prev_p
                                                for d, hj in ((0, h0 % 4), (1, h1 % 4)):
                                                    nc.tensor.matmul(
                                                        pos[d],
                                                        v_g[:, pk, hj * P : (hj + 1) * P],
                                                        pp[d],
                                                        start=(pk == 0),
                                                        stop=False,
                                                        skip_group_check=True,
                                                    )
                                            prev_p = (kc, p_sbs)
                                        pk, pp = prev_p
                                        for d, hj in ((0, h0 % 4), (1, h1 % 4)):
                                            nc.tensor.matmul(
                                                pos[d],
                                                v_g[:, pk, hj * P : (hj + 1) * P],
                                                pp[d],
                                                start=False,
                                                stop=True,
                                                skip_group_check=True,
                                            )
                                        for d, h in ((0, h0), (1, h1)):
                                            zsel = p5ps.tile(
                                                [H, NT], F32, name="zsel",
                                                tag="mm", bufs=6,
                                            )
                                            nc.tensor.matmul(
                                                zsel,
                                                selcol[:, h * H : (h + 1) * H],
                                                accs[d],
                                                start=True,
                                                stop=True,
                                            )
                                            nc.vector.tensor_add(
                                                zrow[:, qcs], zrow[:, qcs], zsel
                                            )
                                            nc.vector.tensor_copy(
                                                ot_tiles[h][:, qcs], pos[d]
                                            )
                            zinv = p5s.tile([H, TQ], F32R, name="zinv")
                            with nc.allow_low_precision(reason="f32r is fp32-width"):
                                nc.vector.reciprocal(zinv, zrow)

                        # -------- deferred softmax normalization ------------
                        with tc.tile_pool(
                            name=f"nrm{rep}", bufs=1, space="PSUM"
                        ) as nrmps:
                            for h in range(H):
                                for qt in range(NQT):
                                    qcs = slice(qt * NT, (qt + 1) * NT)
                                    bc = nrmps.tile(
                                        [P, NT], F32, name="bc", tag="bc", bufs=4
                                    )
                                    nc.tensor.matmul(
                                        bc,
                                        selbrd[:, h * P : (h + 1) * P],
                                        zinv[:, qcs],
                                        start=True,
                                        stop=True,
                                    )
                                    nc.vector.tensor_mul(
                                        ot_tiles[h][:, qcs], ot_tiles[h][:, qcs], bc
                                    )

                        # ================= P6: output projection =============
                        with tc.tile_pool(
                            name=f"p6ps{rep}", bufs=1, space="PSUM"
                        ) as p6ps:
                            for wq in range(4):
                                wo_c = p6w.tile(
                                    [P, NKO * NT], BF16, name="wo_c",
                                    tag="wo_c", bufs=2,
                                )
                                nc.sync.dma_start(wo_c, wo[:, wq, :])
                                pts = {}
                                for mi in range(4):
                                    for q in range(NQT):
                                        pts[(mi, q)] = p6ps.tile(
                                            [P, NT], F32, name="pt",
                                            tag=f"oo{mi}{q}", bufs=1,
                                        )
                                for k in range(NKO):
                                    for mi in range(4):
                                        for q in range(NQT):
                                            nc.tensor.matmul(
                                                pts[(mi, q)],
                                                wo_c[
                                                    :,
                                                    k * NT + mi * P : k * NT
                                                    + (mi + 1) * P,
                                                ],
                                                ot_tiles[k][:, q * NT : (q + 1) * NT],
                                                start=(k == 0),
                                                stop=(k == NKO - 1),
                                            )
                                for mi in range(4):
                                    m = 4 * wq + mi
                                    for q in range(NQT):
                                        sb = p5s.tile(
                                            [P, NT], F32, name="sb", tag="o_sb", bufs=2
                                        )
                                        nc.scalar.activation(sb, pts[(mi, q)], COPY)
                                        nc.sync.dma_start(
                                            outT[
                                                m * P : (m + 1) * P,
                                                q * NT : (q + 1) * NT,
                                            ],
                                            sb,
                                        )

    nc.compile()
    _CACHE[key] = nc
    return nc


# ======================= host-side preparation ===========================


def _deint_perm(d):
    half = d // 2
    perm = np.empty(d, dtype=np.int64)
    perm[:half] = 2 * np.arange(half)
    perm[half:] = 2 * np.arange(half) + 1
    return perm


def _kpanel(w_t):
    """[K, M] -> [P, K//P * M] k-major panels: out[p, k*M + c] = w_t[k*P+p, c]."""
    K, M = w_t.shape
    return np.ascontiguousarray(
        w_t.reshape(K // P, P, M).transpose(1, 0, 2).reshape(P, -1)
    )


def prepare_host_inputs(x, q_a_w, q_a_ln_w, q_b_w, kv_a_w, kv_a_ln_w, kv_b_w, o_w):
    import ml_dtypes

    bf16 = ml_dtypes.bfloat16
    perm = _deint_perm(D_ROPE)

    # ---- q_b: ln folded, per-j blocks [nope0|nope1|pe0:pe1], j-major chunks
    qb = (q_b_w * q_a_ln_w[None, :]).reshape(H, D_QK, Q_LORA)
    qb_rows = np.zeros((NJ, QBJ_COLS, Q_LORA), dtype=np.float32)
    for j in range(NJ):
        h0, h1 = 2 * j, 2 * j + 1
        qb_rows[j, 0:P] = qb[h0, :D_NOPE]
        qb_rows[j, P : 2 * P] = qb[h1, :D_NOPE]
        qb_rows[j, 2 * P : 2 * P + D_ROPE] = qb[h0, D_NOPE:][perm]
        qb_rows[j, 2 * P + D_ROPE : 3 * P] = qb[h1, D_NOPE:][perm]
    wqb_host = np.ascontiguousarray(
        qb_rows.reshape(NJ, QBJ_COLS, NKQ, P).transpose(3, 0, 2, 1).reshape(P, NJ, -1)
    ).astype(bf16)

    # ---- q_a: m-major chunks: wqa[p, m, k*P + c] = q_a_w[m*P+c, k*P+p]
    wqa_host = np.ascontiguousarray(
        q_a_w.reshape(NKQ, P, NKH, P).transpose(3, 0, 2, 1).reshape(P, NKQ, -1)
    ).astype(bf16)

    # ---- kv_a: aug rows [512 ckv | pe(64) | pad(64)], k-major panels
    kva_aug = np.zeros((KVA_COLS, HID), dtype=np.float32)
    kva_aug[:KV_LORA] = kv_a_w[:KV_LORA]
    kva_aug[KV_LORA : KV_LORA + D_ROPE] = kv_a_w[KV_LORA:][perm]
    wkva_host = (
        _kpanel(np.ascontiguousarray(kva_aug.T)).reshape(P, NKH, KVA_COLS).astype(bf16)
    )

    # ---- kv_b: ln folded; split kn / v with chunked layouts
    kvb = (kv_b_w * kv_a_ln_w[None, :]).reshape(H, D_NOPE + D_V, KV_LORA)
    kn_w = kvb[:, :D_NOPE].reshape(H * D_NOPE, KV_LORA).T  # [KV_LORA, H*P]
    v_w = kvb[:, D_NOPE:].reshape(H * D_V, KV_LORA).T  # [KV_LORA, H*D_V]
    # wvb_v[p, g, k, c] = v_w[k*P+p, g*NT+c]
    wvb_v_host = np.ascontiguousarray(
        v_w.reshape(NKV, P, 4, NT).transpose(1, 2, 0, 3)
    ).astype(bf16)
    # wvb_k[p, h, k, c] = kn_w[k*P+p, h*P+c]
    wvb_k_host = np.ascontiguousarray(
        kn_w.reshape(NKV, P, H, P).transpose(1, 2, 0, 3)
    ).astype(bf16)

    # ---- o_w: quarter-major: wo[p, wq, k*NT + mi*P + c] = o_w.T[k*P+p, ...]
    owt = np.ascontiguousarray(o_w.T)  # [H*D_V, HID]
    wo_host = np.ascontiguousarray(
        owt.reshape(NKO, P, 4, 4 * P).transpose(1, 2, 0, 3).reshape(P, 4, -1)
    ).astype(bf16)

    inv_freq = 1.0 / (THETA ** (np.arange(0, D_ROPE, 2, dtype=np.float32) / D_ROPE))
    t = np.arange(S, dtype=np.float32)
    ang = np.outer(inv_freq, t)
    cos = np.concatenate([np.cos(ang)] * 2, axis=0).astype(np.float32)  # [64, S]
    sin = np.concatenate([np.sin(ang)] * 2, axis=0).astype(np.float32)
    cos2 = np.concatenate([cos, cos], axis=0)  # [128, S]
    sin2 = np.concatenate([sin, sin], axis=0)

    selcol_host = np.zeros((P, H * H), dtype=np.float32)
    for h in range(H):
        selcol_host[:, h * H + h] = 1.0
    selbrd_host = np.zeros((H, H * P), dtype=np.float32)
    for h in range(H):
        selbrd_host[h, h * P : (h + 1) * P] = 1.0

    shared = {
        "ones": np.ones((P, 1), dtype=np.float32),
        "selcol": selcol_host.astype(bf16),
        "selbrd": selbrd_host,
        "wqa": wqa_host,
        "wqb": wqb_host,
        "wkva": wkva_host,
        "wvb_v": wvb_v_host,
        "wvb_k": wvb_k_host,
        "wo": wo_host,
    }
    per_core = []
    for c in range(N_CORES):
        b, qh = c // 2, c % 2
        xTb = np.ascontiguousarray(x[b].T)  # [HID, S]
        qs = qh * TQ
        xs = xTb[:, qs : qs + TQ]  # [HID, TQ]
        m = dict(shared)
        m["xq"] = np.ascontiguousarray(
            xs.reshape(NKH, P, TQ).transpose(1, 0, 2).reshape(P, -1)
        ).astype(bf16)
        m["cosq"] = np.ascontiguousarray(cos2[:, qs : qs + TQ]).astype(bf16)
        m["sinq"] = np.ascontiguousarray(sin2[:, qs : qs + TQ]).astype(bf16)
        m["cosk"] = np.ascontiguousarray(cos[:, qs : qs + TQ]).astype(bf16)
        m["sink"] = np.ascontiguousarray(sin[:, qs : qs + TQ]).astype(bf16)
        per_core.append(m)
    return per_core


def kernel(x, q_a_w, q_a_ln_w, q_b_w, kv_a_w, kv_a_ln_w, kv_b_w, o_w):
    from concourse.bass_utils import run_bass_kernel_spmd

    nc = build_nc()
    per_core = prepare_host_inputs(
        np.asarray(x),
        np.asarray(q_a_w),
        np.asarray(q_a_ln_w),
        np.asarray(q_b_w),
        np.asarray(kv_a_w),
        np.asarray(kv_a_ln_w),
        np.asarray(kv_b_w),
        np.asarray(o_w),
    )
    res = run_bass_kernel_spmd(nc, per_core, core_ids=list(range(N_CORES)))
    out = np.empty((B, S, HID), dtype=np.float32)
    for c in range(N_CORES):
        b, qh = c // 2, c % 2
        out[b, qh * TQ : (qh + 1) * TQ] = res.results[c]["outT"].T
    return out
